# revision 1
# baseline (speedup 1.0000x reference)
"""BERT-NER (12-layer BERT-base + token compaction + classifier) on 8 TRN2 cores.

Data-parallel over batch: 16 sequences -> 2 per core. Weights replicated.
Activations are kept feature-major (xT: [768 partitions(6 tiles), 512 tokens])
so every x@W matmul uses the stored W[in,out] directly as lhsT.
All matmuls run in float32r (full PE rate for N>=256), storage stays fp32.
LayerNorm stats, softmax row-sums and free-dim biases are folded into small
matmuls (ones vectors) to avoid partition-dim reductions on DVE.
"""

import os
import sys

for _p in ("/opt/trn_rl_repo", "/root/.axon_site/_ro/trn_rl_repo"):
    if os.path.isdir(_p) and _p not in sys.path:
        sys.path.insert(0, _p)

import numpy as np

import concourse.bass as bass
import concourse.mybir as mybir
import concourse.tile as tile
from concourse.tile import add_dep_helper
from concourse import bacc, bass_utils

F32 = mybir.dt.float32
F32R = mybir.dt.float32r
I32 = mybir.dt.int32
AF = mybir.ActivationFunctionType
ALU = mybir.AluOpType

B, S, H, L, A, V, NL = 16, 256, 768, 12, 12, 30522, 9
DH = H // A  # 64
FF = 4 * H  # 3072
NC = 8  # cores
BL = B // NC  # 2 sequences per core
T = BL * S  # 512 tokens per core
KT = H // 128  # 6 k-tiles of the hidden dim
TC = T // 128  # 4 token chunks
BIG = 1_000_000  # OOB dump index for compaction scatter
EPS = 1e-12
ISCALE = 1.0 / float(np.sqrt(DH))

P = 128


def _r(ap):
    """View an AP as float32r (bit-identical, PE reduced precision)."""
    return ap.bitcast(F32R)


def _f(ap):
    """View an f32r AP as plain float32 for DVE/ACT reads."""
    return ap.bitcast(F32)


def build_nc(repeat=1, n_layers=L):
    nc = bacc.Bacc("TRN2", target_bir_lowering=False, debug=False)

    d_ids = nc.dram_tensor("input_word_ids", [BL, S], I32, kind="ExternalInput")
    d_mask = nc.dram_tensor("input_mask", [BL, S], I32, kind="ExternalInput")
    d_type = nc.dram_tensor("input_type_ids", [BL, S], I32, kind="ExternalInput")
    d_valid = nc.dram_tensor("valid_mask", [BL, S], I32, kind="ExternalInput")
    d_wemb = nc.dram_tensor("word_emb", [V, H], F32, kind="ExternalInput")
    d_pemb = nc.dram_tensor("pos_emb", [S, H], F32, kind="ExternalInput")
    d_temb = nc.dram_tensor("type_emb", [2, H], F32, kind="ExternalInput")
    d_elng = nc.dram_tensor("emb_ln_g", [H], F32, kind="ExternalInput")
    d_elnb = nc.dram_tensor("emb_ln_b", [H], F32, kind="ExternalInput")
    d_Wq = nc.dram_tensor("Wq", [L, H, H], F32, kind="ExternalInput")
    d_bq = nc.dram_tensor("bq", [L, H], F32, kind="ExternalInput")
    d_Wk = nc.dram_tensor("Wk", [L, H, H], F32, kind="ExternalInput")
    d_bk = nc.dram_tensor("bk", [L, H], F32, kind="ExternalInput")
    d_Wv = nc.dram_tensor("Wv", [L, H, H], F32, kind="ExternalInput")
    d_bv = nc.dram_tensor("bv", [L, H], F32, kind="ExternalInput")
    d_Wo = nc.dram_tensor("Wo", [L, H, H], F32, kind="ExternalInput")
    d_bo = nc.dram_tensor("bo", [L, H], F32, kind="ExternalInput")
    d_alg = nc.dram_tensor("attn_ln_g", [L, H], F32, kind="ExternalInput")
    d_alb = nc.dram_tensor("attn_ln_b", [L, H], F32, kind="ExternalInput")
    d_W1 = nc.dram_tensor("W1", [L, H, FF], F32, kind="ExternalInput")
    d_b1 = nc.dram_tensor("b1", [L, FF], F32, kind="ExternalInput")
    d_W2 = nc.dram_tensor("W2", [L, FF, H], F32, kind="ExternalInput")
    d_b2 = nc.dram_tensor("b2", [L, H], F32, kind="ExternalInput")
    d_flg = nc.dram_tensor("ffn_ln_g", [L, H], F32, kind="ExternalInput")
    d_flb = nc.dram_tensor("ffn_ln_b", [L, H], F32, kind="ExternalInput")
    d_clsW = nc.dram_tensor("cls_W", [H, NL], F32, kind="ExternalInput")
    d_clsb = nc.dram_tensor("cls_b", [NL], F32, kind="ExternalInput")
    d_out = nc.dram_tensor("out", [BL, S, NL], F32, kind="ExternalOutput")

    dr = dict(
        ids=d_ids, mask=d_mask, type=d_type, valid=d_valid, wemb=d_wemb,
        pemb=d_pemb, temb=d_temb, elng=d_elng, elnb=d_elnb,
        Wq=d_Wq, bq=d_bq, Wk=d_Wk, bk=d_bk, Wv=d_Wv, bv=d_bv, Wo=d_Wo, bo=d_bo,
        alg=d_alg, alb=d_alb, W1=d_W1, b1=d_b1, W2=d_W2, b2=d_b2,
        flg=d_flg, flb=d_flb, clsW=d_clsW, clsb=d_clsb, out=d_out,
    )

    with nc.allow_low_precision(reason="float32r matmul pipeline"), tile.TileContext(
        nc
    ) as tc:
        with (
            tc.tile_pool(name="const", bufs=1) as cpool,
            tc.tile_pool(name="main", bufs=1) as mpool,
            tc.tile_pool(name="wts", bufs=3) as wpool,
            tc.tile_pool(name="w2p", bufs=4) as w2pool,
            tc.tile_pool(name="hrows", bufs=2) as rpool,
            tc.tile_pool(name="hbuf", bufs=3) as hpool,
            tc.tile_pool(name="ebuf", bufs=4) as epool,
            tc.tile_pool(name="small", bufs=2) as spool,
        ):
            pools = dict(c=cpool, m=mpool, w=wpool, w2=w2pool, r=rpool,
                         h=hpool, e=epool, s=spool)
            # ---- constants (device-generated) ----
            ident = cpool.tile([P, P], F32, tag="ident")
            nc.gpsimd.memset(ident[:], 0.0)
            nc.gpsimd.affine_select(
                out=ident[:], in_=ident[:], compare_op=ALU.not_equal, fill=1.0,
                base=0, pattern=[[-1, P]], channel_multiplier=1,
            )
            ones_f32 = cpool.tile([P, 512], F32, tag="ones_f32")
            nc.gpsimd.memset(ones_f32[:], 1.0)
            ones_row = cpool.tile([1, 512], F32R, tag="ones_row")
            nc.vector.tensor_copy(out=ones_row[:], in_=ones_f32[:1, :])
            ones_col = cpool.tile([P, 1], F32R, tag="ones_col")
            nc.vector.tensor_copy(out=ones_col[:], in_=ones_f32[:, :1])
            ones128 = cpool.tile([P, P], F32R, tag="ones128")
            nc.vector.tensor_copy(out=ones128[:], in_=ones_f32[:, :P])
            # lower-triangular-inclusive: ltri[p, ks, t] = 1 if (ks*128+p) <= t
            ltri_f = cpool.tile([P, 2, S], F32, tag="ltri_f")
            nc.gpsimd.memset(ltri_f[:], 1.0)
            nc.gpsimd.affine_select(
                out=ltri_f[:], in_=ltri_f[:], compare_op=ALU.is_ge, fill=0.0,
                base=0, pattern=[[-P, 2], [1, S]], channel_multiplier=-1,
            )
            c_eps = cpool.tile([1, 1], F32, tag="c_eps")
            nc.gpsimd.memset(c_eps[:], EPS)
            consts = dict(ident=ident, ones_row=ones_row, ones_col=ones_col,
                          ltri=ltri_f, c_eps=c_eps, ones_f32=ones_f32,
                          ones128=ones128)

            def body():
                emit_body(nc, tc, pools, consts, dr, n_layers)

            if repeat == 1:
                body()
            else:
                with tc.For_i(0, repeat, 1):
                    body()

    nc.compile()
    return nc


def _load_w_full(nc, wpool, d_slice):
    """Load a [H, 768] DRAM slice as SBUF [128, KT, 768] (k-tiles on
    partitions). Split across BOTH HWDGE engines (SP + Activation) -- a
    single engine's queue bottlenecks at ~53 GB/s, both reach ~380 GB/s."""
    w = wpool.tile([P, KT, H], F32R, tag="w_big", name="w_big")
    src = d_slice.rearrange("(kt p) c -> p kt c", p=P)
    nc.sync.dma_start(w[:, 0:3], _r(src[:, 0:3]))
    nc.scalar.dma_start(w[:, 3:6], _r(src[:, 3:6]))
    return w


def _bias_col(nc, spool, d_vec, tag):
    """Load [H] DRAM vector as [128, KT] (col m = slice m*128:(m+1)*128)."""
    t = spool.tile([P, KT], F32, tag=tag, name=tag)
    nc.sync.dma_start(t[:], d_vec.rearrange("(kt p) -> p kt", p=P))
    return t


def _bias_row(nc, rpool, d_vec, tag="brow", dtype=F32R):
    """Load a DRAM vector [N<=768] as a single-partition row [1, N]."""
    n = d_vec.shape[0]
    t = rpool.tile([1, n], dtype, tag=tag, name=tag)
    nc.sync.dma_start(t[:], d_vec[None, :].bitcast(dtype))
    return t


def emit_ln(nc, tc, mpool, spool, y, g_col, b_col, out_tag, consts):
    """Feature-major layernorm. Stat matmuls use an all-ones [128,128] lhsT so
    the per-token sums land already broadcast across 128 partitions; the whole
    stats chain then runs 128-lane on DVE/ACT."""
    ones128 = consts["ones128"]
    c_eps = consts["c_eps"]
    out = mpool.tile([P, KT, 512], F32R, tag=out_tag, name=out_tag)
    with tc.tile_pool(name="lnps", bufs=1, space="PSUM") as ppool:
        ps_s1 = ppool.tile([P, 512], F32, tag="ln_s1", space="PSUM")
        ps_s2 = ppool.tile([P, 512], F32, tag="ln_s2", space="PSUM")
        for kt in range(KT):
            sq = mpool.tile([P, 512], F32R, tag="ln_sq", bufs=2, name="sq")
            nc.scalar.activation(sq[:], _f(y[:, kt]), AF.Square)
            nc.tensor.matmul(ps_s1[:], ones128[:], y[:, kt],
                             start=(kt == 0), stop=(kt == KT - 1))
            nc.tensor.matmul(ps_s2[:], ones128[:], sq[:],
                             start=(kt == 0), stop=(kt == KT - 1))
        mean = spool.tile([P, 512], F32, tag="ln_mean", bufs=1, name="ln_mean")
        nc.vector.tensor_scalar_mul(mean[:], ps_s1[:], 1.0 / H)
        m2 = spool.tile([P, 512], F32, tag="ln_m2", bufs=1, name="ln_m2")
        nc.vector.tensor_tensor(out=m2[:], in0=mean[:], in1=mean[:], op=ALU.mult)
        var = spool.tile([P, 512], F32, tag="ln_var", bufs=1, name="ln_var")
        nc.vector.tensor_scalar(out=var[:], in0=ps_s2[:], scalar1=1.0 / H,
                                scalar2=EPS, op0=ALU.mult, op1=ALU.add)
        nc.vector.tensor_tensor(out=var[:], in0=var[:], in1=m2[:], op=ALU.subtract)
        std = spool.tile([P, 512], F32, tag="ln_std", bufs=1, name="ln_std")
        nc.scalar.activation(std[:], var[:], AF.Sqrt)
        rstd = spool.tile([P, 512], F32, tag="ln_rstd", bufs=1, name="ln_rstd")
        nc.vector.reciprocal(rstd[:], std[:])
        for kt in range(KT):
            tmp = mpool.tile([P, 512], F32, tag="ln_tmp", bufs=2, name="tmp")
            nc.vector.tensor_tensor(out=tmp[:], in0=_f(y[:, kt]), in1=mean[:],
                                    op=ALU.subtract)
            nc.vector.tensor_tensor(out=tmp[:], in0=tmp[:], in1=rstd[:],
                                    op=ALU.mult)
            nc.scalar.activation(out[:, kt], tmp[:], AF.Identity,
                                 scale=g_col[:, kt : kt + 1],
                                 bias=b_col[:, kt : kt + 1])
    return out


def emit_body(nc, tc, pools, consts, dr, n_layers):
    cpool, mpool, wpool, w2pool = (
        pools["c"], pools["m"], pools["w"], pools["w2"])
    rpool, hpool, epool, spool = (
        pools["r"], pools["h"], pools["e"], pools["s"])
    ident, ones_row, ones_col, ltri = (
        consts["ident"], consts["ones_row"], consts["ones_col"], consts["ltri"])
    ones_f32 = consts["ones_f32"]

    ids_flat = dr["ids"].rearrange("b s -> (b s)")
    type_flat = dr["type"].rearrange("b s -> (b s)")
    mask_flat = dr["mask"].rearrange("b s -> (b s)")
    valid_flat = dr["valid"].rearrange("b s -> (b s)")

    # amask[:, c]: 0 where mask==1 else -10000 ; valid_f: valid mask as f32r
    amask = cpool.tile([P, TC], F32, tag="amask", name="amask")
    valid_f = cpool.tile([P, TC], F32, tag="valid_f", name="valid_f")

    # ============ embeddings (token-major), transpose, LN ============
    xtok = mpool.tile([P, TC, H], F32, tag="bigA", name="xtok")
    for c in range(TC):
        idt = spool.tile([P, 1], I32, tag="idt", name="idt")
        nc.sync.dma_start(idt[:], ids_flat[c * P : (c + 1) * P, None])
        nc.gpsimd.indirect_dma_start(
            out=xtok[:, c], out_offset=None, in_=dr["wemb"][:, :],
            in_offset=bass.IndirectOffsetOnAxis(ap=idt[:, :1], axis=0),
        )
        tyt = spool.tile([P, 1], I32, tag="tyt", name="tyt")
        nc.sync.dma_start(tyt[:], type_flat[c * P : (c + 1) * P, None])
        temb = hpool.tile([P, H], F32, tag="temb", bufs=2, name="temb")
        nc.gpsimd.indirect_dma_start(
            out=temb[:], out_offset=None, in_=dr["temb"][:, :],
            in_offset=bass.IndirectOffsetOnAxis(ap=tyt[:, :1], axis=0),
        )
        pemb = hpool.tile([P, H], F32, tag="pemb", bufs=2, name="pemb")
        cc = c % (S // P)
        nc.sync.dma_start(pemb[:], dr["pemb"][cc * P : (cc + 1) * P, :])
        nc.vector.tensor_tensor(out=xtok[:, c], in0=xtok[:, c], in1=temb[:],
                                op=ALU.add)
        nc.vector.tensor_tensor(out=xtok[:, c], in0=xtok[:, c], in1=pemb[:],
                                op=ALU.add)

        mi = spool.tile([P, 1], I32, tag="mi", name="mi")
        nc.sync.dma_start(mi[:], mask_flat[c * P : (c + 1) * P, None])
        mf = spool.tile([P, 1], F32, tag="mf", name="mf")
        nc.vector.tensor_copy(out=mf[:], in_=mi[:])
        nc.scalar.activation(amask[:, c : c + 1], mf[:], AF.Copy,
                             scale=10000.0, bias=-10000.0)
        vi = spool.tile([P, 1], I32, tag="vi", name="vi")
        nc.sync.dma_start(vi[:], valid_flat[c * P : (c + 1) * P, None])
        nc.vector.tensor_copy(out=valid_f[:, c : c + 1], in_=vi[:])

    # transpose to feature-major
    xe = mpool.tile([P, KT, 512], F32R, tag="bigB", name="xe")
    with tc.tile_pool(name="embtr", bufs=2, space="PSUM") as ppool:
        for kt in range(KT):
            for c in range(TC):
                ps_t = ppool.tile([P, P], F32, tag="tr", space="PSUM")
                nc.tensor.transpose(
                    out=ps_t[:], in_=xtok[:, c, kt * P : (kt + 1) * P],
                    identity=ident[:])
                nc.vector.tensor_copy(out=xe[:, kt, c * P : (c + 1) * P],
                                      in_=ps_t[:])
    eg = _bias_col(nc, spool, dr["elng"][:], "eg")
    eb = _bias_col(nc, spool, dr["elnb"][:], "eb")
    x = emit_ln(nc, tc, mpool, spool, xe, eg, eb, "x_cur", consts)

    # ============ transformer layers ============
    for l in range(n_layers):
        # ---- Q, K projections (feature-major out) ----
        qT = mpool.tile([P, KT, 512], F32R, tag="bigA", name="qT")
        kT = mpool.tile([P, KT, 512], F32R, tag="bigB", name="kT")
        vsb = mpool.tile([P, TC, A, DH], F32R, tag="vsb", name="vsb")
        with tc.tile_pool(name=f"qkv{l}", bufs=2, space="PSUM") as ppool:
            for name, d_W, d_b, dst in (("q", dr["Wq"], dr["bq"], qT),
                                        ("k", dr["Wk"], dr["bk"], kT)):
                bc = _bias_col(nc, spool, d_b[l], f"b_{name}")
                w = _load_w_full(nc, wpool, d_W[l])
                for m in range(KT):
                    ps = ppool.tile([P, 512], F32, tag="qk_ps", space="PSUM")
                    for kt in range(KT):
                        nc.tensor.matmul(ps[:],
                                         w[:, kt, m * P : (m + 1) * P],
                                         x[:, kt], start=(kt == 0),
                                         stop=(kt == KT - 1))
                    nc.scalar.activation(dst[:, m], ps[:], AF.Identity,
                                         bias=bc[:, m : m + 1])
            # ---- V (token-major, with per-head ones column) ----
            bvr = _bias_row(nc, rpool, dr["bv"][l])
            wv = _load_w_full(nc, wpool, dr["Wv"][l])
            for hh in range(2):
                for c in range(TC):
                    ps = ppool.tile([P, 384], F32, tag="v_ps", space="PSUM")
                    for kt in range(KT):
                        nc.tensor.matmul(
                            ps[:], x[:, kt, c * P : (c + 1) * P],
                            wv[:, kt, hh * 384 : (hh + 1) * 384],
                            start=(kt == 0), stop=False)
                    nc.tensor.matmul(ps[:], ones_row[:1, :P],
                                     bvr[:1, hh * 384 : (hh + 1) * 384],
                                     start=False, stop=True)
                    nc.vector.tensor_copy(
                        out=vsb[:, c, hh * 6 : (hh + 1) * 6, :],
                        in_=ps[:].rearrange("p (h d) -> p h d", d=DH))

        # ---- attention per (seq, head) ----
        ctxT = mpool.tile([P, KT, 512], F32R, tag="bigC", name="ctxT")
        with tc.tile_pool(name=f"att{l}", bufs=2, space="PSUM") as ppool:
            for b in range(BL):
                for h in range(A):
                    ft, fo = h // 2, (h % 2) * DH
                    ps_s = ppool.tile([P, 2, S], F32, tag="s_ps", bufs=3,
                                      space="PSUM")
                    esb = []
                    for kc in range(2):
                        nc.tensor.matmul(
                            ps_s[:, kc],
                            kT[fo : fo + DH, ft,
                               b * S + kc * P : b * S + (kc + 1) * P],
                            qT[fo : fo + DH, ft, b * S : (b + 1) * S],
                            start=True, stop=True)
                        e = epool.tile([P, S], F32R, tag="e_sb", bufs=6, name="e_sb")
                        nc.scalar.activation(
                            e[:], ps_s[:, kc], AF.Exp, scale=ISCALE,
                            bias=amask[:, b * 2 + kc : b * 2 + kc + 1])
                        esb.append(e)
                    # row-sums of E (tiny ones-column matmul) off the ctx path
                    ps_sm = ppool.tile([1, S], F32, tag="sm_ps", bufs=1,
                                       space="PSUM")
                    for kc in range(2):
                        nc.tensor.matmul(ps_sm[:], ones_col[:], esb[kc][:],
                                         start=(kc == 0), stop=(kc == 1))
                    rsum = spool.tile([1, S], F32R, tag="rsum", bufs=4, name="rsum")
                    nc.vector.reciprocal(rsum[:], ps_sm[:])
                    ps_b = ppool.tile([DH, S], F32, tag="b_ps", bufs=1,
                                      space="PSUM")
                    nc.tensor.matmul(ps_b[:], ones_row[:1, :DH], rsum[:],
                                     start=True, stop=True)
                    bsb = epool.tile([DH, S], F32, tag="bsb", bufs=4, name="bsb")
                    nc.vector.tensor_copy(out=bsb[:], in_=ps_b[:])
                    ps_c = ppool.tile([DH, S], F32, tag="c_ps", bufs=3, space="PSUM")
                    for kc in range(2):
                        nc.tensor.matmul(ps_c[:], vsb[:, b * 2 + kc, h],
                                         esb[kc][:], start=(kc == 0),
                                         stop=(kc == 1))
                    nc.vector.tensor_tensor(
                        out=ctxT[fo : fo + DH, ft, b * S : (b + 1) * S],
                        in0=ps_c[:], in1=bsb[:], op=ALU.mult)

        # ---- output projection + residual + LN1 ----
        y1 = mpool.tile([P, KT, 512], F32R, tag="bigA", name="y1")
        with tc.tile_pool(name=f"wo{l}", bufs=2, space="PSUM") as ppool:
            bor = _bias_row(nc, rpool, dr["bo"][l])
            wo = _load_w_full(nc, wpool, dr["Wo"][l])
            for m in range(KT):
                    ps = ppool.tile([P, 512], F32, tag="o_ps", space="PSUM")
                    for kt in range(KT):
                        nc.tensor.matmul(ps[:], wo[:, kt, m * P : (m + 1) * P],
                                         ctxT[:, kt], start=(kt == 0), stop=False)
                    nc.tensor.matmul(ps[:], bor[:1, m * P : (m + 1) * P],
                                     ones_row[:1, :], start=False, stop=True)
                    nc.vector.tensor_tensor(out=y1[:, m], in0=ps[:],
                                            in1=_f(x[:, m]), op=ALU.add)
            ag = _bias_col(nc, spool, dr["alg"][l], "ag")
            ab = _bias_col(nc, spool, dr["alb"][l], "ab")
            x2 = emit_ln(nc, tc, mpool, spool, y1, ag, ab, "bigB", consts)

        # ---- FFN (y2 accumulated in 6 persistent PSUM banks) ----
        y3 = mpool.tile([P, KT, 512], F32R, tag="bigC", name="y3")
        with (
            tc.tile_pool(name=f"ffa{l}", bufs=1, space="PSUM") as papool,
            tc.tile_pool(name=f"ffh{l}", bufs=2, space="PSUM") as ppool,
        ):
            b2r = _bias_row(nc, rpool, dr["b2"][l])
            ps_y = [papool.tile([P, 512], F32, tag=f"acc{m}",
                                name=f"ps_y{l}_{m}", space="PSUM")
                    for m in range(KT)]
            for q4 in range(4):
                w1 = _load_w_full(nc, wpool, dr["W1"][l][:, q4 * H : (q4 + 1) * H])
                b1r = _bias_row(nc, rpool, dr["b1"][l][q4 * H : (q4 + 1) * H])
                for cc in range(KT):
                    c = q4 * KT + cc
                    ps_h = ppool.tile([P, 512], F32, tag="h_ps", space="PSUM")
                    for kt in range(KT):
                        nc.tensor.matmul(ps_h[:],
                                         w1[:, kt, cc * P : (cc + 1) * P],
                                         x2[:, kt], start=(kt == 0), stop=False)
                    nc.tensor.matmul(ps_h[:],
                                     b1r[:1, (c % KT) * P : (c % KT + 1) * P],
                                     ones_row[:1, :], start=False, stop=True)
                    hsb = hpool.tile([P, 512], F32R, tag="h_sb", name="hsb")
                    nc.scalar.activation(hsb[:], ps_h[:], AF.Gelu_apprx_tanh)
                    w2 = w2pool.tile([P, H], F32R, tag="w2c", name="w2")
                    nc.sync.dma_start(w2[:],
                                      _r(dr["W2"][l][c * P : (c + 1) * P, :]))
                    for m in range(KT):
                        nc.tensor.matmul(ps_y[m][:], w2[:, m * P : (m + 1) * P],
                                         hsb[:], start=(c == 0), stop=False)
            for m in range(KT):
                nc.tensor.matmul(ps_y[m][:], b2r[:1, m * P : (m + 1) * P],
                                 ones_row[:1, :], start=False, stop=True)
                nc.vector.tensor_tensor(out=y3[:, m], in0=ps_y[m][:],
                                        in1=_f(x2[:, m]), op=ALU.add)
        fg = _bias_col(nc, spool, dr["flg"][l], "fg")
        fb = _bias_col(nc, spool, dr["flb"][l], "fb")
        x = emit_ln(nc, tc, mpool, spool, y3, fg, fb, "x_cur", consts)

    # ============ classifier head + softmax + compaction ============
    with tc.tile_pool(name="head", bufs=2, space="PSUM") as ppool:
        clsw = spool.tile([P, KT, NL], F32, tag="clsw", name="clsw")
        nc.sync.dma_start(clsw[:], dr["clsW"].rearrange("(kt p) c -> p kt c", p=P))
        clsb = _bias_row(nc, rpool, dr["clsb"][:], tag="clsb", dtype=F32)

        # uniform pad row: softmax(cls_b), broadcast to 128 partitions
        nmx = spool.tile([1, 1], F32, tag="nmx", name="nmx")
        nc.vector.reduce_max(out=nmx[:], in_=clsb[:], negate=True,
                             axis=mybir.AxisListType.X)
        usum = spool.tile([1, 1], F32, tag="usum", name="usum")
        uex = spool.tile([1, NL], F32, tag="uex", name="uex")
        nc.scalar.activation(uex[:], clsb[:], AF.Exp, bias=nmx[:],
                             accum_out=usum[:])
        urs = spool.tile([1, 1], F32, tag="urs", name="urs")
        nc.vector.reciprocal(urs[:], usum[:])
        uni = spool.tile([1, NL], F32, tag="uni", name="uni")
        nc.vector.tensor_scalar_mul(uni[:], uex[:], urs[:])
        ps_u = ppool.tile([P, NL], F32, tag="u_ps", space="PSUM")
        nc.tensor.matmul(ps_u[:], ones_f32[:1, :P], uni[:], start=True, stop=True)
        uni128 = spool.tile([P, NL], F32, tag="uni128", name="uni128")
        nc.vector.tensor_copy(out=uni128[:], in_=ps_u[:])
        out_flat = dr["out"].rearrange("b s c -> (b s) c")
        prefills = []
        for c in range(TC):
            dma = nc.sync.dma_start(out_flat[c * P : (c + 1) * P, :], uni128[:])
            prefills.append(dma.ins)

        for c in range(TC):
            b = c // (S // P)
            ps_lg = ppool.tile([P, NL], F32, tag="lg_ps", space="PSUM")
            for kt in range(KT):
                nc.tensor.matmul(ps_lg[:], _f(x[:, kt, c * P : (c + 1) * P]),
                                 clsw[:, kt], start=(kt == 0), stop=False)
            nc.tensor.matmul(ps_lg[:], ones_f32[:1, :P], clsb[:],
                             start=False, stop=True)
            negmax = spool.tile([P, 1], F32, tag="negmax", name="negmax")
            nc.vector.reduce_max(out=negmax[:], in_=ps_lg[:], negate=True,
                                 axis=mybir.AxisListType.X)
            probs = spool.tile([P, NL], F32, tag="probs", name="probs")
            sm = spool.tile([P, 1], F32, tag="sm", name="sm")
            nc.scalar.activation(probs[:], ps_lg[:], AF.Exp, bias=negmax[:],
                                 accum_out=sm[:])
            rs = spool.tile([P, 1], F32, tag="rs", name="rs")
            nc.vector.reciprocal(rs[:], sm[:])
            nc.vector.tensor_scalar_mul(probs[:], probs[:], rs[:])

            # cumsum of valid over the sequence, sliced to this chunk
            cc = c % (S // P)
            ps_cs = ppool.tile([P, 1], F32, tag="cs_ps", space="PSUM")
            for ks in range(2):
                nc.tensor.matmul(ps_cs[:], ltri[:, ks, cc * P : (cc + 1) * P],
                                 valid_f[:, b * 2 + ks : b * 2 + ks + 1],
                                 start=(ks == 0), stop=(ks == 1))
            # dest = valid ? b*S + csum - 1 : BIG
            dest_f = spool.tile([P, 1], F32, tag="dest_f", name="dest_f")
            nc.vector.tensor_scalar_add(dest_f[:], ps_cs[:], float(b * S - 1 - BIG))
            nc.vector.tensor_tensor(out=dest_f[:], in0=dest_f[:],
                                    in1=valid_f[:, c : c + 1], op=ALU.mult)
            nc.vector.tensor_scalar_add(dest_f[:], dest_f[:], float(BIG))
            dest_i = spool.tile([P, 1], I32, tag="dest_i", name="dest_i")
            nc.vector.tensor_copy(out=dest_i[:], in_=dest_f[:])

            scat = nc.gpsimd.indirect_dma_start(
                out=out_flat[:, :],
                out_offset=bass.IndirectOffsetOnAxis(ap=dest_i[:, :1], axis=0),
                in_=probs[:],
                in_offset=None,
                bounds_check=T - 1, oob_is_err=False,
            )
            for pf in prefills:
                add_dep_helper(scat.ins, pf,
                               reason="scatter after uniform prefill")


_NC_CACHE = {}


def _get_nc(repeat=1, n_layers=L):
    key = (repeat, n_layers)
    if key not in _NC_CACHE:
        _NC_CACHE[key] = build_nc(repeat=repeat, n_layers=n_layers)
    return _NC_CACHE[key]


def make_in_maps(inputs):
    per_seq = {}
    for name in ("input_word_ids", "input_mask", "input_type_ids", "valid_mask"):
        per_seq[name] = np.ascontiguousarray(np.asarray(inputs[name]))
    shared = {
        k: np.ascontiguousarray(np.asarray(v))
        for k, v in inputs.items()
        if k not in per_seq
    }
    in_maps = []
    for c in range(NC):
        m = dict(shared)
        for name, arr in per_seq.items():
            m[name] = np.ascontiguousarray(arr[c * BL : (c + 1) * BL])
        in_maps.append(m)
    return in_maps


def kernel(**inputs):
    nc = _get_nc()
    in_maps = make_in_maps(inputs)
    res = bass_utils.run_bass_kernel_spmd(nc, in_maps, list(range(NC)))
    out = np.concatenate([res.results[c]["out"] for c in range(NC)], axis=0)
    return out.astype(np.float32)

